# revision 1
# baseline (speedup 1.0000x reference)
"""Cdist-mean kernel for Trainium2 (8 NeuronCores, SPMD row-sharded).

Computes mean(cdist(x.reshape(T,-1), y.reshape(T,-1))) for T=8192, D=512.

Sharding: core c gets x rows [c*1024, (c+1)*1024) and all of y (the TxT
distance matrix is row-sharded); each core returns per-partition partial
sums which the host adds and divides by T^2.

Per core, sq[i,j] = x2[i] + y2[j] - 2*x.y with K on PSUM partitions:
  - operands arrive via HWDGE DMA-transpose (bf16) on two queues (x on
    the scalar queue, y on sync, chunked so early segments land first),
    then are cast once to fp8 e4m3 on DVE
  - x.y: 2 fp8 DoubleRow matmuls (K=256 each) accumulate into PSUM at
    2x the bf16 rate
  - one K=128-padded bf16 matmul adds -x2[i]/2 - y2[j]/2 in the same
    accumulation group (padding lets its LDWEIGHTS hide like the mains;
    rows 2..127 of both operands are zero).  x2/y2 rows are built on
    device with ones-matmuls over squared operands; a per-partition
    scale/bias DVE op writes [1.0 ; -y2/2] rows exactly
  - ACT: sqrt(-2*psum) over multi-bank PSUM groups with accum_out doing
    the free-dim sum reduction inside the same instruction
  - y2 prep is issued just-in-time per segment so the PE FIFO never
    blocks on a not-yet-DMA'd chunk

Numerics: fp8 only touches the cross term (zero-mean rounding); x2/y2
come from bf16 squares in f32 PSUM; final accumulation is f32 on chip
and f64 on host.  End-to-end relative error ~1e-4.
"""

import sys

import numpy as np

if "/opt/trn_rl_repo" not in sys.path:
    sys.path.insert(0, "/opt/trn_rl_repo")

import ml_dtypes

T = 8192
D = 512  # flattened feature dim (256*2)
NCORES = 8
M = T // NCORES  # 1024 rows of x per core
P = 128
KC = D // P  # 4 K-chunks
MT = M // P  # 8 m-tiles per core
SEG = 512  # n-segment (matmul free dim)
NSEG = T // SEG  # 16

_CACHE = {}


def _build():
    import concourse.bass as bass
    import concourse.tile as tile
    from concourse import bacc, mybir

    nc = bacc.Bacc(
        "TRN2",
        target_bir_lowering=False,
        debug=False,
        enable_asserts=False,
        num_devices=NCORES,
    )

    xs = nc.dram_tensor("xs", [M, D], mybir.dt.bfloat16, kind="ExternalInput").ap()
    yb = nc.dram_tensor("yb", [T, D], mybir.dt.bfloat16, kind="ExternalInput").ap()
    out = nc.dram_tensor(
        "out", [P, 72], mybir.dt.float32, kind="ExternalOutput"
    ).ap()

    with tile.TileContext(nc) as tc:
        with (
            tc.tile_pool(name="persist", bufs=1) as persist,
            tc.tile_pool(name="work", bufs=8) as work,
            tc.tile_pool(name="psum", bufs=3, space="PSUM") as pp,
            tc.tile_pool(name="psum_y2", bufs=2, space="PSUM") as pp_y2,
        ):
            f32 = mybir.dt.float32
            bf16 = mybir.dt.bfloat16

            # ---- persistent tiles ----
            yt = persist.tile([P, KC, T], bf16, tag="yt")
            xt = persist.tile([P, KC, M], bf16, tag="xt")
            # aug rhs, K padded to 128 so its LDWEIGHTS hides like the main
            # matmuls': row0 = ones, row1 = -y2[j]/2, rows 2..127 = 0
            aug = persist.tile([P, T], bf16, tag="aug")
            # aug lhsT: row0 = -x2[m]/2, row1 = ones, rows 2..127 = 0
            augL = persist.tile([P, M], bf16, tag="augL")
            acc_cols = persist.tile([P, 72], f32, tag="acc_cols")
            ones_col2 = persist.tile([P, 2], bf16, tag="ones_col2")
            # per-partition scale/bias for the y2 ACT: row0 = 0*in+1 = 1.0,
            # row1 = -0.5*in + 0 = -y2/2
            sc_y2 = persist.tile([2, 1], f32, tag="sc_y2")
            bi_y2 = persist.tile([2, 1], f32, tag="bi_y2")

            nc.vector.memset(ones_col2[:], 1.0)
            nc.gpsimd.memset(aug[:], 0.0)
            nc.vector.memset(augL[:], 0.0)
            nc.vector.memset(augL[0:2, :], 1.0)
            nc.vector.memset(sc_y2[:], -0.5)
            nc.vector.memset(sc_y2[0:1, :], 0.0)
            nc.vector.memset(bi_y2[:], 0.0)
            nc.vector.memset(bi_y2[0:1, :], 1.0)

            f8 = mybir.dt.float8e4
            # fp8 copies of the transposed operands for DoubleRow matmuls
            yt8 = persist.tile([P, KC, T], f8, tag="yt8")
            xt8 = persist.tile([P, KC, M], f8, tag="xt8")

            # ---- transposes: xt on the scalar HWDGE queue, y on sync, so
            # the two streams overlap and the first main group starts early
            # xt[kc][k, m] = x[m, kc*128+k]
            for kc in range(KC):
                nc.scalar.dma_start_transpose(
                    xt[:, kc, :], xs[:, kc * P : (kc + 1) * P]
                )
            nc.vector.tensor_copy(xt8[:], xt[:])
            y_chunks = [(0, 512), (512, 512), (1024, 1024), (2048, 1536), (3584, 1536), (5120, 1536), (6656, 1536)]
            for q0, qw in y_chunks:
                for kc in range(KC):
                    nc.sync.dma_start_transpose(
                        yt[:, kc, q0 : q0 + qw],
                        yb[q0 : q0 + qw, kc * P : (kc + 1) * P],
                    )

            # ---- x2 row: augL[0, m] = -x2[m]/2 via ones-matmul over xt^2
            # (issued after the first y2_preps so the prologue DVE FIFO
            # prioritizes what the first main matmuls need) ----
            def x2_prep():
                for h in range(M // SEG):
                    ps_x2 = pp_y2.tile([2, SEG], f32, tag="ps_y2", name="ps_x2")
                    seg = xt[:, :, h * SEG : (h + 1) * SEG]
                    xsq = work.tile([P, KC, SEG], bf16, tag="ysq", name="xsq")
                    nc.vector.tensor_tensor(xsq[:], seg, seg, mybir.AluOpType.mult)
                    for kc in range(KC):
                        nc.tensor.matmul(
                            ps_x2[0:1, :],
                            ones_col2[:, 0:1],
                            xsq[:, kc, :],
                            start=(kc == 0),
                            stop=(kc == KC - 1),
                        )
                    nc.scalar.activation(
                        augL[0:1, h * SEG : (h + 1) * SEG],
                        ps_x2[0:1, :],
                        mybir.ActivationFunctionType.Copy,
                        scale=-0.5,
                    )

            # y2 prep for one segment: aug[0, j] = -y2[j]/2 (bf16).
            # Issued just-in-time inside the main loop so a y2 matmul for a
            # not-yet-DMA'd segment never blocks resident main matmuls in
            # the PE's FIFO queue.
            def y2_prep(s):
                ps_y2 = pp_y2.tile([2, SEG], f32, tag="ps_y2", name="ps_y2")
                seg = yt[:, :, s * SEG : (s + 1) * SEG]
                # fp8 copy for the DoubleRow mains + squares for y2, one 3D
                # DVE op each (just-in-time so the DVE FIFO never blocks on
                # a not-yet-DMA'd chunk)
                nc.vector.tensor_copy(yt8[:, :, s * SEG : (s + 1) * SEG], seg)
                ysq = work.tile([P, KC, SEG], bf16, tag="ysq", name="ysq")
                nc.vector.tensor_tensor(ysq[:], seg, seg, mybir.AluOpType.mult)
                for kc in range(KC):
                    nc.tensor.matmul(
                        ps_y2[:],
                        ones_col2[:],
                        ysq[:, kc, :],
                        start=(kc == 0),
                        stop=(kc == KC - 1),
                    )
                # per-partition scale/bias on DVE (keeps ACT free for sqrt):
                # row0 = 0*in + 1 = 1.0 exactly, row1 = -0.5*in + 0 = -y2/2
                nc.vector.tensor_scalar(
                    aug[0:2, s * SEG : (s + 1) * SEG],
                    ps_y2[:],
                    sc_y2[:],
                    bi_y2[:],
                    mybir.AluOpType.mult,
                    mybir.AluOpType.add,
                )

            # ---- main loop: several segments share one multi-bank PSUM
            # tile so a single ACT sqrt (+accum) covers them all ----
            GROUPS = [1, 1, 2, 2, 2, 2, 2, 2, 2]  # seg counts; 2 banks x 3 bufs + 2 = 8
            GMAX = max(GROUPS)
            col = 0
            s0 = 0
            for nb, gn in enumerate(GROUPS):
                for g in range(gn):
                    y2_prep(s0 + g)
                if nb == 0:
                    x2_prep()
                for mi in range(MT):
                    psum = pp.tile([P, GMAX * SEG], f32, tag="psum", name="psum")
                    for g in range(gn):
                        ni = s0 + g
                        sub = psum[:, g * SEG : (g + 1) * SEG]
                        for c2 in range(KC // 2):
                            nc.tensor.matmul(
                                sub,
                                xt8[:, 2 * c2 : 2 * c2 + 2, mi * P : (mi + 1) * P],
                                yt8[:, 2 * c2 : 2 * c2 + 2, ni * SEG : (ni + 1) * SEG],
                                start=(c2 == 0),
                                stop=False,
                                perf_mode=mybir.MatmulPerfMode.DoubleRow,
                            )
                        nc.tensor.matmul(
                            sub,
                            augL[:, mi * P : (mi + 1) * P],
                            aug[:, ni * SEG : (ni + 1) * SEG],
                            start=False,
                            stop=True,
                        )
                    nc.scalar.activation(
                        psum[:, : gn * SEG],
                        psum[:, : gn * SEG],
                        mybir.ActivationFunctionType.Sqrt,
                        scale=-2.0,
                        accum_out=acc_cols[:, col : col + 1],
                    )
                    col += 1
                s0 += gn

            nc.sync.dma_start(out[:], acc_cols[:])

    nc.compile()
    return nc


def _get_nc():
    if "nc" not in _CACHE:
        _CACHE["nc"] = _build()
    return _CACHE["nc"]


def _run(x, y, trace=False, **kw):
    from concourse.bass_utils import run_bass_kernel_spmd

    xf = np.ascontiguousarray(np.asarray(x, dtype=np.float32).reshape(T, D))
    yf = np.ascontiguousarray(np.asarray(y, dtype=np.float32).reshape(T, D))
    xb = xf.astype(ml_dtypes.bfloat16)
    ybv = yf.astype(ml_dtypes.bfloat16)

    nc = _get_nc()
    in_maps = [
        {"xs": np.ascontiguousarray(xb[c * M : (c + 1) * M]), "yb": ybv}
        for c in range(NCORES)
    ]
    res = run_bass_kernel_spmd(
        nc, in_maps, core_ids=list(range(NCORES)), trace=trace, **kw
    )
    total = sum(float(r["out"].astype(np.float64).sum()) for r in res.results)
    val = np.float32(total / (float(T) * float(T)))
    return np.array(val, dtype=np.float32), res


def kernel(x, y):
    out, _ = _run(x, y)
    return out

